# revision 7
# baseline (speedup 1.0000x reference)
"""AdaptiveConv2DMod Trainium2 kernel.

Per-sample modulated 3x3 conv (StyleGAN2-style) on 8 NeuronCores,
data-parallel over batch (1 sample per core, no collectives).

Per-core layout:
  - Input rows stream through a circular SBUF "tape": image row r lives at
    partition group r%4 (32 channels each), slot (r//4)%NSLOT, width padded
    to W+2 with zero columns.
  - Conv = 3 x-taps (kx) x row-window matmuls: for each output row y, the
    contraction over (in-channel, ky) is a K<=96 matmul over partition
    groups holding rows y-1..y+1.  Rows 4t..4t+3 form 4 PSUM col groups of
    one PSUM bank -> one [128, W] evacuation copy per 4 rows.
  - Weights (per-sample softmax-mixed + modulated + demodulated) are built
    on-device, transposed to lhsT layout via PE transposes, and replicated
    into 4 row-alignment variants so any (row-window, tile_position) pair
    reads one contiguous AP.
  - Matmuls run as float32r (TF32-like): full-rate at N=512, ample accuracy.
"""

import os
import sys

import numpy as np

try:
    import concourse.bass as bass  # noqa: F401
except ImportError:
    sys.path.insert(0, "/opt/trn_rl_repo")

import concourse.bass as bass
import concourse.tile as tile
from concourse import bacc, mybir
from concourse.bass_utils import run_bass_kernel_spmd

F32 = mybir.dt.float32
F32R = mybir.dt.float32r
BF16 = mybir.dt.bfloat16

C = 32          # in/out channels
NK = 4          # kernel bank size
EPS = 1e-8


def build_graph(H=512, W=512, nslot=32):
    """Build the per-core Bass graph. Returns compiled Bacc."""
    T = H // 4                      # row groups
    nslot = min(nslot, T)
    Wp = W + 2                      # padded width

    nc = bacc.Bacc("TRN2", target_bir_lowering=False, debug=False)

    fmap = nc.dram_tensor("fmap", [C, H, W], F32, kind="ExternalInput")
    mod = nc.dram_tensor("mod", [1, C], F32, kind="ExternalInput")
    kmod = nc.dram_tensor("kernel_mod", [1, NK], F32, kind="ExternalInput")
    wbank = nc.dram_tensor("weights", [NK, C, C, 3, 3], F32, kind="ExternalInput")
    ident = nc.dram_tensor("ident32", [C, C], F32, kind="ExternalInput")
    out = nc.dram_tensor("out", [C, H, W], F32, kind="ExternalOutput")

    # DRAM views: rows 4t+g at partition 32g+i
    fm_v = fmap.ap().rearrange("i (t g) x -> t g i x", g=4)
    out_v = out.ap().rearrange("o (t g) x -> t g o x", g=4)

    def pieces_for_row(y):
        # maximal same-slot runs, split to K<=64 at even-strip boundaries so
        # every piece is a legal tile_position (32@{0,32,64,96}, 64@{0,64})
        lo, hi = max(y - 1, 0), min(y + 1, H - 1)
        runs = []
        r = lo
        while r <= hi:
            e = min(hi, (r // 4) * 4 + 3)
            if r % 4 in (1, 3):
                runs.append((r, r))
                r += 1
                continue
            if e > r:
                runs.append((r, r + 1))
                r += 2
            else:
                runs.append((r, r))
                r += 1
        return runs

    with tile.TileContext(nc) as tc:
        # ---------------- persistent pools ----------------
        with (
            tc.tile_pool(name="xpool", bufs=1) as xpool,
            tc.tile_pool(name="wpool", bufs=1) as wpool,
            tc.tile_pool(name="cpool", bufs=1) as cpool,
        ):
            X = xpool.tile([128, nslot * Wp], BF16)
            Wt = wpool.tile([128, 384], BF16)      # 4 alignments x 3 kx x 32 o
            id32 = cpool.tile([C, C], F32)
            ones1 = cpool.tile([1, C], F32)

            nc.sync.dma_start(id32[:, :], ident.ap())
            nc.gpsimd.memset(ones1[:, :], 1.0)

            # zero the pad columns of every slot (cols 0 and Wp-1)
            xv = X[:, :].rearrange("p (s q) -> p s q", q=Wp)
            nc.vector.memset(xv[:, :, 0], 0.0)
            nc.vector.memset(xv[:, :, Wp - 1], 0.0)

            # ---------------- weight preparation ----------------
            with (
                tc.tile_pool(name="prep", bufs=2) as prep,
                tc.tile_pool(name="prep_ps", bufs=2, space="PSUM") as prep_ps,
            ):
                # softmax(kernel_mod) -> attn [1, NK]
                km = prep.tile([1, NK], F32)
                nc.sync.dma_start(km[:, :], kmod.ap())
                mx = prep.tile([1, 1], F32)
                nc.vector.reduce_max(mx[:, :], km[:, :], axis=mybir.AxisListType.X)
                nmx = prep.tile([1, 1], F32)
                nc.scalar.mul(nmx[:, :], mx[:, :], -1.0)
                ex = prep.tile([1, NK], F32)
                nc.scalar.activation(
                    ex[:, :], km[:, :], mybir.ActivationFunctionType.Exp,
                    bias=nmx[:, 0:1],
                )
                sm = prep.tile([1, 1], F32)
                nc.vector.reduce_sum(sm[:, :], ex[:, :], axis=mybir.AxisListType.X)
                rs = prep.tile([1, 1], F32)
                nc.vector.reciprocal(rs[:, :], sm[:, :])
                attn = prep.tile([1, NK], F32)
                nc.vector.tensor_scalar_mul(attn[:, :], ex[:, :], rs[:, 0:1])

                # broadcast attn to all 32 partitions
                attnB = prep.tile([C, NK], F32)
                nc.gpsimd.partition_broadcast(attnB[:, :], attn[:, :])

                # P[o, n*288 + i*9 + tap] = weights[n, o, i, ky, kx]
                P = prep.tile([C, NK * 288], F32)
                nc.sync.dma_start(
                    P[:, :], wbank.ap().rearrange("n o i ky kx -> o n (i ky kx)")
                )

                # mix[o, i*9+tap] = sum_n attn[n] * P[o, n, ...]
                mix = prep.tile([C, 288], F32, tag="mix")
                tmp = prep.tile([C, 288], F32, tag="tmp")
                nc.vector.tensor_scalar_mul(mix[:, :], P[:, 0:288], attnB[:, 0:1])
                for n in range(1, NK):
                    nc.vector.tensor_scalar_mul(
                        tmp[:, :], P[:, n * 288:(n + 1) * 288], attnB[:, n:n + 1]
                    )
                    nc.vector.tensor_add(mix[:, :], mix[:, :], tmp[:, :])

                # mvec[i, 1] = mod + 1 ;  m2 = mvec^2
                mv = prep.tile([C, 1], F32, tag="mv")
                nc.sync.dma_start(mv[:, :], mod.ap().rearrange("a i -> i a"))
                m1 = prep.tile([C, 1], F32, tag="m1")
                nc.scalar.add(m1[:, :], mv[:, :], 1.0)
                m2 = prep.tile([C, 1], F32, tag="m2")
                nc.vector.tensor_mul(m2[:, :], m1[:, :], m1[:, :])

                # s[o, i] = sum_tap mix^2
                sq = prep.tile([C, 288], F32, tag="tmp")
                nc.vector.tensor_mul(sq[:, :], mix[:, :], mix[:, :])
                s_oi = prep.tile([C, C], F32, tag="soi")
                nc.vector.reduce_sum(
                    s_oi[:, :],
                    sq[:, :].rearrange("p (i t) -> p i t", t=9),
                    axis=mybir.AxisListType.X,
                )
                # sT[i, o]
                ps_a = prep_ps.tile([C, C], F32, tag="psa")
                nc.tensor.transpose(ps_a[:, :], s_oi[:, :], id32[:, :])
                sT = prep.tile([C, C], F32, tag="soi")
                nc.vector.tensor_copy(sT[:, :], ps_a[:, :])

                # normsq[1, o] = m2 . sT  (contract i)
                ps_n = prep_ps.tile([1, C], F32, tag="psa")
                nc.tensor.matmul(
                    ps_n[:, :], m2[:, :], sT[:, :], start=True, stop=True
                )
                ns = prep.tile([1, C], F32, tag="ns")
                nc.vector.tensor_scalar_max(ns[:, :], ps_n[:, :], EPS)
                sqn = prep.tile([1, C], F32, tag="sqn")
                nc.scalar.sqrt(sqn[:, :], ns[:, :])
                inv = prep.tile([1, C], F32, tag="inv")
                nc.vector.reciprocal(inv[:, :], sqn[:, :])

                # invT[o, 1] via PE transpose (identity [1,1] = ones)
                ps_i = prep_ps.tile([C, 1], F32, tag="psa")
                nc.tensor.transpose(ps_i[:, :], inv[:, :], ones1[:, 0:1])
                invT = prep.tile([C, 1], F32, tag="invT")
                nc.vector.tensor_copy(invT[:, :], ps_i[:, :])

                # wtA[o, i*9+tap] = mix * inv[o]
                wtA = prep.tile([C, 288], F32, tag="mix2")
                nc.vector.tensor_scalar_mul(wtA[:, :], mix[:, :], invT[:, 0:1])

                # 9 PE transposes -> psW[i, tap*32+o]
                ps_w = prep_ps.tile([C, 288], F32, tag="psw")
                wtA_t = wtA[:, :].rearrange("p (i t) -> p t i", t=9)
                for tap in range(9):
                    nc.tensor.transpose(
                        ps_w[:, tap * C:(tap + 1) * C], wtA_t[:, tap, :], id32[:, :]
                    )
                # wtB0[i, ky*96 + kx*32 + o] = psW * (1+mod[i])
                wtB0 = prep.tile([C, 288], BF16, tag="wtB0")
                nc.vector.tensor_scalar_mul(wtB0[:, :], ps_w[:, :], m1[:, 0:1])

                # replicate into 4 alignment variants:
                # Wt[32g+i, 96a + kxo] = wtB0[i, 96*((g-a)%4) + kxo]
                for a in range(4):
                    for ky in range(3):
                        g = (a + ky) % 4
                        nc.sync.dma_start(
                            Wt[32 * g:32 * g + 32, 96 * a:96 * a + 96],
                            wtB0[:, 96 * ky:96 * ky + 96],
                        )

            # ---------------- main conv loop ----------------
            with (
                tc.tile_pool(name="cps", bufs=4, space="PSUM") as cps,
                tc.tile_pool(name="opool", bufs=4) as opool,
                tc.tile_pool(name="spool", bufs=4) as spool,
            ):
                def dma_in(t):
                    s = t % nslot
                    stg = spool.tile([128, W], F32, tag="stg")
                    nc.sync.dma_start(stg[:, :], fm_v[t])
                    # f32 -> bf16 rounding pass (DVE)
                    nc.vector.tensor_copy(
                        X[:, s * Wp + 1: s * Wp + 1 + W], stg[:, :]
                    )

                dma_in(0)
                for t in range(T):
                    # group t reads rows 4t-1 .. 4t+4 -> needs slot t+1 loaded
                    if t + 1 < T:
                        dma_in(t + 1)
                    pt = cps.tile([128, W], F32, tag="pt")
                    # col groups must open/close sequentially within a bank:
                    # finish all of c's matmuls before starting c+1 (the HW
                    # start=True bank clear + sim group tracker require it).
                    # PE still overlaps columns: c+1's streams use different
                    # column strips and start while c's are draining.
                    for c in range(4):
                        y = 4 * t + c
                        a = (y - 1) % 4
                        runs = pieces_for_row(y)
                        started = False
                        for kx in range(3):
                            for (rstart, rend) in runs:
                                K = 32 * (rend - rstart + 1)
                                g0 = rstart % 4
                                sl = (rstart // 4) % nslot
                                lhsT = Wt[
                                    32 * g0:32 * g0 + K,
                                    96 * a + 32 * kx:96 * a + 32 * kx + 32,
                                ]
                                rhs = X[
                                    32 * g0:32 * g0 + K,
                                    sl * Wp + kx: sl * Wp + kx + W,
                                ]
                                last = (kx == 2) and ((rstart, rend) == runs[-1])
                                nc.tensor.matmul(
                                    pt[32 * c:32 * c + 32, :],
                                    lhsT,
                                    rhs,
                                    start=not started,
                                    stop=last,
                                    tile_position=(32 * g0, 32 * c),
                                )
                                started = True
                    ot = opool.tile([128, W], F32, tag="ot")
                    nc.scalar.copy(ot[:, :], pt[:, :])
                    nc.sync.dma_start(out_v[t], ot[:, :])

    nc.compile()
    return nc


_CACHE = {}


def _get_graph(H, W):
    key = (H, W)
    if key not in _CACHE:
        _CACHE[key] = build_graph(H, W)
    return _CACHE[key]


def kernel(fmap, mod, kernel_mod, weights):
    B, Ci, H, Wd = fmap.shape
    nc = _get_graph(H, Wd)
    eye = np.eye(C, dtype=np.float32)
    in_maps = [
        {
            "fmap": np.ascontiguousarray(fmap[b], dtype=np.float32),
            "mod": np.ascontiguousarray(mod[b:b + 1], dtype=np.float32),
            "kernel_mod": np.ascontiguousarray(kernel_mod[b:b + 1], dtype=np.float32),
            "weights": np.ascontiguousarray(weights, dtype=np.float32),
            "ident32": eye,
        }
        for b in range(B)
    ]
    res = run_bass_kernel_spmd(nc, in_maps, core_ids=list(range(B)))
    return np.stack([res.results[b]["out"] for b in range(B)], axis=0)
